# revision 27
# baseline (speedup 1.0000x reference)
"""Trainium2 Bass kernel for nn_AttentionLSTM (B=8, S=256, D=256, N=256).

Math:
  Wx  = X @ Wx_w.T + Wx_b            [B,S,N]
  Wxh = X @ Wxhat_w.T + Wxhat_b      [B,S,N]
  A   = sigmoid(tanh(Wxh[:,None,:,:] + Wx[:,:,None,:]) @ att_w + att_b)  [B,S,S]
  out = A @ X                         [B,S,D]

Strategy: data-parallel over batch (1 batch per NeuronCore, 8 cores).
The [S,S,N] tanh tensor is never materialized: tanh(t) ~ sum_m K_m
sin(2^m a0 t), each sine separating via angle addition into two rank-N
matmuls, 24 bf16 MMs total on the TensorEngine.

Structure (one act-table load for the whole kernel, set 18 has sin+tanh
+square):
- cb = Wx_b + Wxhat_b is folded into the T2 PSUM by one extra rank-1
  accumulating matmul per nt (stationary row 0 = cb, moving = ones), so
  all four ACT seeds are fused [128, 2, S] ops with constant/no bias.
- Only s/c come from the sin table; sin(2a0 z) = s*c and sin(4a0 z) =
  (s*c)*c1 ladders on DVE with the 2x/4x absorbed into fold coefs.
  The ladder squares (q1, k2, k1) run on ACT (Square) in its idle
  window between the seeds and the output activations.
- The output sigmoid is 0.5 + 0.5*tanh(z/2): tanh is in the sin set, so
  no second table load.  The affine is exact: X is pre-halved on the
  host (XH2 = X/2) and the 0.5*colsum(X) term is one extra
  ones-stationary matmul accumulated into the output PSUM.
- Input DMAs split across both HWDGE rings (SP + ACT) pairwise in need
  order; gpsimd SWDGE only carries memsets and the late xh1 load (its
  elementwise ops contend with DVE on the shared SBUF port - measured
  2-3x slowdown on both - so everything pointwise stays on DVE).
- Dummy/bridge matmuls keep the PE HAM activity window continuously
  busy so the clock un-throttles to 2.4 GHz before the attention
  bursts (v1 measured no un-throttle at all: every MM ran at 1.2 GHz).
"""

from contextlib import ExitStack

import math

import ml_dtypes
import numpy as np

import concourse.bacc as bacc
import concourse.bass as bass
import concourse.mybir as mybir
import concourse.tile as tile
from concourse.bass_utils import run_bass_kernel_spmd

F32 = mybir.dt.float32
BF16 = mybir.dt.bfloat16
AF = mybir.ActivationFunctionType
OP = mybir.AluOpType

B, S, D, N = 8, 256, 256, 256
NCORES = 8
P = 128

# tanh(t) ~ K0 sin(a0 t) + K1 sin(2 a0 t) + K2 sin(4 a0 t), end-to-end LSQ
A0 = 0.42
COEFS = (1.287571, 0.032552, 0.223318)

_nc_cache = {}


def _build_nc():
    if "nc" in _nc_cache:
        return _nc_cache["nc"]
    nc = bacc.Bacc()

    xt_d = nc.declare_dram_parameter("XT", [D, S], BF16, isOutput=False)
    xh_d = nc.declare_dram_parameter("XH2", [S, D], BF16, isOutput=False)  # X/2
    w1t_d = nc.declare_dram_parameter("W1T", [D, N], BF16, isOutput=False)
    w2t_d = nc.declare_dram_parameter("W2T", [D, N], BF16, isOutput=False)
    cb1_d = nc.declare_dram_parameter("CB1", [1, N], BF16, isOutput=False)
    # packed per-partition constants, cols:
    #   0:2   ws1[nt] = K0*att_w
    #   2:4   ws2[nt] = 2*K1*att_w
    #   4:6   -2*ws2[nt]
    #   6:8   ws3[nt] = 4*K2*att_w
    #   8:10  2*ws3[nt]
    #   10:12 -ws3[nt]
    #   12    att_b/2   (bias for the tanh-form sigmoid)
    #   13    pi/2
    cw_d = nc.declare_dram_parameter("CW", [P, 14], F32, isOutput=False)
    out_d = nc.declare_dram_parameter("out", [S, D], BF16, isOutput=True)

    with tile.TileContext(nc) as tc, ExitStack() as ctx:
        sb = ctx.enter_context(tc.tile_pool(name="sb", bufs=1))
        ps = ctx.enter_context(tc.tile_pool(name="ps", bufs=1, space="PSUM"))

        # Set 18 = silu_and_others: sin + tanh + square. The only load.
        nc.scalar.add_instruction(
            mybir.InstLoadActFuncSet(
                act_func_set_id=18, name=nc.get_next_instruction_name()
            )
        )

        xt = [sb.tile([P, S], BF16, tag=f"xt{i}", name=f"xt{i}") for i in range(2)]
        xh = [sb.tile([P, D], BF16, tag=f"xh{i}", name=f"xh{i}") for i in range(2)]
        w1t = [sb.tile([P, N], BF16, tag=f"w1t{i}", name=f"w1t{i}") for i in range(2)]
        w2t = [sb.tile([P, N], BF16, tag=f"w2t{i}", name=f"w2t{i}") for i in range(2)]
        cw = sb.tile([P, 14], F32, tag="cw", name="cw")
        cbT = sb.tile([P, N], BF16, tag="cbT", name="cbT")
        ones = sb.tile([P, S], BF16, tag="ones", name="ones")
        dmy = sb.tile([P, 4 * P], BF16, tag="dmy", name="dmy")

        # gpsimd: memsets (cbT before its CB1 row DMA) + small SWDGE loads.
        nc.gpsimd.memset(dmy[:], 0.0)
        nc.gpsimd.memset(cbT[:], 0.0)
        nc.gpsimd.dma_start(out=cbT[0:1, :], in_=cb1_d[:, :])
        nc.gpsimd.memset(ones[:], 1.0)
        nc.gpsimd.dma_start(out=xh[1][:], in_=xh_d[P : 2 * P, :])

        # SP HWDGE ring, in need order.
        nc.sync.dma_start(out=xt[0][:], in_=xt_d[0:P, :])
        nc.sync.dma_start(out=w2t[1][:], in_=w2t_d[P : 2 * P, :])
        nc.sync.dma_start(out=w1t[1][:], in_=w1t_d[P : 2 * P, :])
        nc.sync.dma_start(out=cw[:], in_=cw_d[:, :])
        nc.sync.dma_start(out=xh[0][:], in_=xh_d[0:P, :])

        # ACT HWDGE ring, in parallel: the partner operand of each pair.
        nc.scalar.dma_start(out=w2t[0][:], in_=w2t_d[0:P, :])
        nc.scalar.dma_start(out=xt[1][:], in_=xt_d[P : 2 * P, :])
        nc.scalar.dma_start(out=w1t[0][:], in_=w1t_d[0:P, :])

        dps = ps.tile([P, 4 * P], F32, tag="dps", name="dps")
        for i in range(6):
            nc.tensor.matmul(
                dps[:], dmy[:, 0:P], dmy[:], start=(i == 0), stop=False,
                skip_group_check=True,
            )

        # ---- projections, dt-major to match DMA arrival order.  T2 and T1
        # in SEPARATE PSUM tiles: tile-granular dependency tracking must not
        # make the T2 seeds wait for the T1 writers.
        pp2 = ps.tile([P, 2, S], F32, tag="pp2", name="pp2")
        pp1 = ps.tile([P, 2, S], F32, tag="pp1", name="pp1")
        for nt in range(2):
            for dt in range(2):
                nc.tensor.matmul(
                    pp2[:, nt, :],
                    w2t[dt][:, nt * P : (nt + 1) * P],
                    xt[dt][:],
                    start=(dt == 0),
                    stop=False,
                    skip_group_check=True,
                )
            nc.tensor.matmul(
                pp2[:, nt, :],
                cbT[:, nt * P : (nt + 1) * P],
                ones[:],
                start=False,
                stop=True,
                skip_group_check=True,
            )
        for nt in range(2):
            for dt in range(2):
                nc.tensor.matmul(
                    pp1[:, nt, :],
                    w1t[dt][:, nt * P : (nt + 1) * P],
                    xt[dt][:],
                    start=(dt == 0),
                    stop=(dt == 1),
                    skip_group_check=True,
                )

        # ---- seeds straight from PSUM, T2 first (feeds the longer chain)
        s2t = sb.tile([P, 2, S], BF16, tag="s2t", name="s2t")
        c2t = sb.tile([P, 2, S], BF16, tag="c2t", name="c2t")
        s1t = sb.tile([P, 2, S], BF16, tag="s1t", name="s1t")
        c1t = sb.tile([P, 2, S], BF16, tag="c1t", name="c1t")
        nc.scalar.activation(s2t[:], pp2[:], AF.Sin, scale=A0)
        nc.scalar.activation(c2t[:], pp2[:], AF.Sin, bias=cw[:, 13:14], scale=A0)
        nc.scalar.activation(s1t[:], pp1[:], AF.Sin, scale=A0)
        nc.scalar.activation(c1t[:], pp1[:], AF.Sin, bias=cw[:, 13:14], scale=A0)

        # ---- ladder + folds on DVE, in first-MM-need order.  The 4a0
        # cos factors come from cos(4fz) = 1 - 8 h^2 (h = sin(2fz)/2), so
        # there is no cross-engine round-trip; only q1 = s1^2 runs on ACT
        # (its idle window after the seeds).
        q2 = sb.tile([P, 2, S], BF16, tag="q2", name="q2")
        h2 = sb.tile([P, 2, S], BF16, tag="h2", name="h2")
        c12 = sb.tile([P, 2, S], BF16, tag="c12", name="c12")
        u2 = sb.tile([P, 2, S], BF16, tag="u2", name="u2")
        g2 = sb.tile([P, 2, S], BF16, tag="g2", name="g2")
        q1 = sb.tile([P, 2, S], BF16, tag="q1", name="q1")
        h1 = sb.tile([P, 2, S], BF16, tag="h1", name="h1")
        c11 = sb.tile([P, 2, S], BF16, tag="c11", name="c11")
        u1 = sb.tile([P, 2, S], BF16, tag="u1", name="u1")
        g1 = sb.tile([P, 2, S], BF16, tag="g1", name="g1")
        c21 = sb.tile([P, 2, S], BF16, tag="c21", name="c21")
        fc0 = sb.tile([P, 2, S], BF16, tag="fc0", name="fc0")
        fp0 = sb.tile([P, 2, S], BF16, tag="fp0", name="fp0")
        fp1 = sb.tile([P, 2, S], BF16, tag="fp1", name="fp1")
        fc1 = sb.tile([P, 2, S], BF16, tag="fc1", name="fc1")
        fp2 = sb.tile([P, 2, S], BF16, tag="fp2", name="fp2")
        fc2 = sb.tile([P, 2, S], BF16, tag="fc2", name="fc2")

        V = nc.vector
        for nt in range(2):  # tier-1 folds first: earliest MM operands
            V.tensor_scalar_mul(fc0[:, nt, :], s2t[:, nt, :], cw[:, 0 + nt : 1 + nt])
        V.tensor_mul(q2[:], s2t[:], s2t[:])
        for nt in range(2):
            V.tensor_scalar_mul(fp0[:, nt, :], c2t[:, nt, :], cw[:, 0 + nt : 1 + nt])
        V.tensor_mul(h2[:], s2t[:], c2t[:])
        for nt in range(2):  # fp1 = ws2*(1-2 q2), fused
            V.tensor_scalar(
                fp1[:, nt, :], q2[:, nt, :],
                cw[:, 4 + nt : 5 + nt], cw[:, 2 + nt : 3 + nt],
                OP.mult, OP.add,
            )
        V.tensor_scalar(c12[:], q2[:], -2.0, 1.0, OP.mult, OP.add)
        for nt in range(2):
            V.tensor_scalar_mul(fc1[:, nt, :], h2[:, nt, :], cw[:, 2 + nt : 3 + nt])
        V.tensor_mul(q1[:], s1t[:], s1t[:])
        V.tensor_mul(h1[:], s1t[:], c1t[:])
        V.tensor_scalar(c11[:], q1[:], -2.0, 1.0, OP.mult, OP.add)
        V.tensor_mul(u2[:], h2[:], h2[:])
        V.tensor_mul(g2[:], h2[:], c12[:])
        for nt in range(2):  # fp2 = K2*aw*(1 - 8 u2), fused
            V.tensor_scalar(
                fp2[:, nt, :], u2[:, nt, :],
                cw[:, 8 + nt : 9 + nt], cw[:, 10 + nt : 11 + nt],
                OP.mult, OP.add,
            )
        for nt in range(2):
            V.tensor_scalar_mul(fc2[:, nt, :], g2[:, nt, :], cw[:, 6 + nt : 7 + nt])
        V.tensor_mul(u1[:], h1[:], h1[:])
        V.tensor_mul(g1[:], h1[:], c11[:])
        V.tensor_scalar(c21[:], u1[:], -8.0, 1.0, OP.mult, OP.add)

        # Keep-busy bridges anchored on early-ready seeds fill the PE
        # operand-wait gap between the projections and the first burst.
        def bridge(anchor):
            nc.tensor.matmul(
                dps[:], anchor[:, 0, 0:P], dmy[:], start=False, stop=False,
                skip_group_check=True,
            )

        bridge(s2t)
        bridge(s2t)
        bridge(c2t)

        # ---- attention logits Apre^T[j,i]: 6 product pairs x 2 nt x 2 jt
        ap_ps = [
            ps.tile([P, S], F32, tag=f"apre{jt}", name=f"apre{jt}") for jt in range(2)
        ]
        pairs = (
            (fp0, s1t), (fc0, c1t),
            (fp1, h1), (fc1, c11),
            (fp2, g1), (fc2, c21),
        )
        for k, (stat, mov) in enumerate(pairs):
            for nt in range(2):
                for jt in range(2):
                    idx = k * 2 + nt
                    nc.tensor.matmul(
                        ap_ps[jt][:],
                        stat[:, nt, jt * P : (jt + 1) * P],
                        mov[:, nt, :],
                        start=(idx == 0),
                        stop=(idx == 11),
                        skip_group_check=True,
                    )

        # A^T = 0.5 + 0.5*tanh(z/2 + att_b/2); the tanh part goes through
        # the output matmul against XH2 = X/2, the 0.5 offset becomes a
        # ones-stationary colsum matmul.  Column halves so the first
        # output matmul can start after two ACT ops.
        at = [sb.tile([P, S], BF16, tag=f"at{jt}", name=f"at{jt}") for jt in range(2)]
        for jt in range(2):
            nc.scalar.activation(
                at[jt][:],
                ap_ps[jt][:],
                AF.Tanh,
                bias=cw[:, 12:13],
                scale=0.5,
            )

        # ---- out[i,d] = sum_j (0.5 + 0.5 th^T[j,i]) * X[j,d]
        #              = sum_j ones*XH2 + sum_j th^T[j,i]*XH2[j,d]
        for it in range(2):
            o_ps = ps.tile([P, D], F32, tag=f"ops{it}", name=f"ops{it}")
            for jt in range(2):
                nc.tensor.matmul(
                    o_ps[:],
                    ones[:, it * P : (it + 1) * P],
                    xh[jt][:],
                    start=(jt == 0),
                    stop=False,
                    skip_group_check=True,
                )
            for jt in range(2):
                nc.tensor.matmul(
                    o_ps[:],
                    at[jt][:, it * P : (it + 1) * P],
                    xh[jt][:],
                    start=False,
                    stop=(jt == 1),
                    skip_group_check=True,
                )
            oc = sb.tile([P, D], BF16, tag=f"oc{it}", name=f"oc{it}")
            if it == 0:
                nc.vector.tensor_copy(oc[:], o_ps[:])
                nc.sync.dma_start(out=out_d[0:P, :], in_=oc[:])
            else:
                nc.scalar.activation(oc[:], o_ps[:], AF.Copy)
                nc.scalar.dma_start(out=out_d[P : 2 * P, :], in_=oc[:])

    nc.finalize()
    _nc_cache["nc"] = nc
    return nc


def _host_prep(X, Wx_w, Wx_b, Wxhat_w, Wxhat_b, att_w, att_b):
    bf = ml_dtypes.bfloat16
    w1t = np.ascontiguousarray(Wx_w.T).astype(bf)
    w2t = np.ascontiguousarray(Wxhat_w.T).astype(bf)
    cb1 = (Wx_b + Wxhat_b).astype(np.float32).reshape(1, N).astype(bf)
    aw = att_w.astype(np.float32)
    ws = [COEFS[0] * aw, 2 * COEFS[1] * aw, 4 * COEFS[2] * aw]
    cw = np.zeros((P, 14), np.float32)
    for nt in range(2):
        sl = slice(nt * P, (nt + 1) * P)
        cw[:, 0 + nt] = ws[0][sl]
        cw[:, 2 + nt] = ws[1][sl]
        cw[:, 4 + nt] = -2.0 * ws[1][sl]
        cw[:, 6 + nt] = ws[2][sl]
        cw[:, 8 + nt] = -8.0 * ws[2][sl]  # fp2 = ws3*(1-8u2), ws3 = 4 K2 aw
        cw[:, 10 + nt] = ws[2][sl]        # (pairs with g1 = sin(4 a0 x)/4)
    cw[:, 12] = 0.5 * float(np.asarray(att_b).reshape(-1)[0])
    cw[:, 13] = np.pi / 2
    shared = {"W1T": w1t, "W2T": w2t, "CW": cw, "CB1": cb1}
    in_maps = []
    for b in range(B):
        xb = np.ascontiguousarray(X[b], dtype=np.float32)
        in_maps.append(
            {
                "XH2": (0.5 * xb).astype(bf),
                "XT": np.ascontiguousarray(xb.T).astype(bf),
                **shared,
            }
        )
    return in_maps


def run(inputs, trace=False):
    nc = _build_nc()
    in_maps = _host_prep(**inputs)
    res = run_bass_kernel_spmd(nc, in_maps, core_ids=list(range(NCORES)), trace=trace)
    out = np.stack(
        [np.asarray(res.results[i]["out"]).astype(np.float32) for i in range(NCORES)],
        axis=0,
    )
    return out, res.exec_time_ns


def kernel(**inputs):
    out, _ = run(inputs, trace=False)
    return out


# revision 30
# speedup vs baseline: 1.0924x; 1.0924x over previous
"""Trainium2 Bass kernel for nn_AttentionLSTM (B=8, S=256, D=256, N=256).

Math:
  Wx  = X @ Wx_w.T + Wx_b            [B,S,N]
  Wxh = X @ Wxhat_w.T + Wxhat_b      [B,S,N]
  A   = sigmoid(tanh(Wxh[:,None,:,:] + Wx[:,:,None,:]) @ att_w + att_b)  [B,S,S]
  out = A @ X                         [B,S,D]

Strategy: data-parallel over batch (1 batch per NeuronCore, 8 cores).
The [S,S,N] tanh tensor is never materialized: tanh(t) ~ sum_m K_m
sin(2^m a0 t), each sine separating via angle addition into two rank-N
matmuls, 24 bf16 MMs total on the TensorEngine.

Structure (one act-table load for the whole kernel, set 18 has sin+tanh
+square):
- cb = Wx_b + Wxhat_b is folded into the T2 PSUM by one extra rank-1
  accumulating matmul per nt (stationary row 0 = cb, moving = ones), so
  all four ACT seeds are fused [128, 2, S] ops with constant/no bias.
- Only s/c come from the sin table; sin(2a0 z) = s*c and sin(4a0 z) =
  (s*c)*c1 ladders on DVE with the 2x/4x absorbed into fold coefs.
  The ladder squares (q1, k2, k1) run on ACT (Square) in its idle
  window between the seeds and the output activations.
- The output sigmoid is 0.5 + 0.5*tanh(z/2): tanh is in the sin set, so
  no second table load.  The affine is exact: X is pre-halved on the
  host (XH2 = X/2) and the 0.5*colsum(X) term is one extra
  ones-stationary matmul accumulated into the output PSUM.
- Input DMAs split across both HWDGE rings (SP + ACT) pairwise in need
  order; gpsimd SWDGE only carries memsets and the late xh1 load (its
  elementwise ops contend with DVE on the shared SBUF port - measured
  2-3x slowdown on both - so everything pointwise stays on DVE).
- Dummy/bridge matmuls keep the PE HAM activity window continuously
  busy so the clock un-throttles to 2.4 GHz before the attention
  bursts (v1 measured no un-throttle at all: every MM ran at 1.2 GHz).
"""

from contextlib import ExitStack

import math

import ml_dtypes
import numpy as np

import concourse.bacc as bacc
import concourse.bass as bass
import concourse.mybir as mybir
import concourse.tile as tile
from concourse.bass_utils import run_bass_kernel_spmd

F32 = mybir.dt.float32
BF16 = mybir.dt.bfloat16
AF = mybir.ActivationFunctionType
OP = mybir.AluOpType

B, S, D, N = 8, 256, 256, 256
NCORES = 8
P = 128

# tanh(t) ~ K0 sin(a0 t) + K1 sin(2 a0 t), end-to-end LSQ
A0 = 0.42
COEFS = (0.11045, 0.951604)

_nc_cache = {}


def _build_nc():
    if "nc" in _nc_cache:
        return _nc_cache["nc"]
    nc = bacc.Bacc()

    xt_d = nc.declare_dram_parameter("XT", [D, S], BF16, isOutput=False)
    xh_d = nc.declare_dram_parameter("XH2", [S, D], BF16, isOutput=False)  # X/2
    w1t_d = nc.declare_dram_parameter("W1T", [D, N], BF16, isOutput=False)
    w2t_d = nc.declare_dram_parameter("W2T", [D, N], BF16, isOutput=False)
    cb1_d = nc.declare_dram_parameter("CB1", [1, N], BF16, isOutput=False)
    # packed per-partition constants, cols:
    #   0:2   ws1[nt] = K0*att_w
    #   2:4   ws2[nt] = 2*K1*att_w
    #   4:6   -2*ws2[nt]
    #   6:8   ws3[nt] = 4*K2*att_w
    #   8:10  2*ws3[nt]
    #   10:12 -ws3[nt]
    #   12    att_b/2   (bias for the tanh-form sigmoid)
    #   13    pi/2
    cw_d = nc.declare_dram_parameter("CW", [P, 14], F32, isOutput=False)
    out_d = nc.declare_dram_parameter("out", [S, D], BF16, isOutput=True)

    with tile.TileContext(nc) as tc, ExitStack() as ctx:
        sb = ctx.enter_context(tc.tile_pool(name="sb", bufs=1))
        ps = ctx.enter_context(tc.tile_pool(name="ps", bufs=1, space="PSUM"))

        # Set 18 = silu_and_others: sin + tanh + square. The only load.
        nc.scalar.add_instruction(
            mybir.InstLoadActFuncSet(
                act_func_set_id=18, name=nc.get_next_instruction_name()
            )
        )

        xt = [sb.tile([P, S], BF16, tag=f"xt{i}", name=f"xt{i}") for i in range(2)]
        xh = [sb.tile([P, D], BF16, tag=f"xh{i}", name=f"xh{i}") for i in range(2)]
        w1t = [sb.tile([P, N], BF16, tag=f"w1t{i}", name=f"w1t{i}") for i in range(2)]
        w2t = [sb.tile([P, N], BF16, tag=f"w2t{i}", name=f"w2t{i}") for i in range(2)]
        cw = sb.tile([P, 14], F32, tag="cw", name="cw")
        cbT = sb.tile([P, N], BF16, tag="cbT", name="cbT")
        ones = sb.tile([P, S], BF16, tag="ones", name="ones")
        dmy = sb.tile([P, 4 * P], BF16, tag="dmy", name="dmy")

        # gpsimd: memsets (cbT before its CB1 row DMA) + small SWDGE loads.
        nc.gpsimd.memset(dmy[:], 0.0)
        nc.gpsimd.memset(cbT[:], 0.0)
        nc.gpsimd.dma_start(out=cbT[0:1, :], in_=cb1_d[:, :])
        nc.gpsimd.memset(ones[:], 1.0)
        nc.gpsimd.dma_start(out=xh[1][:], in_=xh_d[P : 2 * P, :])

        # SP HWDGE ring, in need order.
        nc.sync.dma_start(out=xt[0][:], in_=xt_d[0:P, :])
        nc.sync.dma_start(out=w2t[1][:], in_=w2t_d[P : 2 * P, :])
        nc.sync.dma_start(out=w1t[1][:], in_=w1t_d[P : 2 * P, :])
        nc.sync.dma_start(out=cw[:], in_=cw_d[:, :])
        nc.sync.dma_start(out=xh[0][:], in_=xh_d[0:P, :])

        # ACT HWDGE ring, in parallel: the partner operand of each pair.
        nc.scalar.dma_start(out=w2t[0][:], in_=w2t_d[0:P, :])
        nc.scalar.dma_start(out=xt[1][:], in_=xt_d[P : 2 * P, :])
        nc.scalar.dma_start(out=w1t[0][:], in_=w1t_d[0:P, :])

        dps = ps.tile([P, 4 * P], F32, tag="dps", name="dps")
        for i in range(6):
            nc.tensor.matmul(
                dps[:], dmy[:, 0:P], dmy[:], start=(i == 0), stop=False,
                skip_group_check=True,
            )

        # ---- projections, dt-major to match DMA arrival order.  T2 and T1
        # in SEPARATE PSUM tiles: tile-granular dependency tracking must not
        # make the T2 seeds wait for the T1 writers.
        pp2 = ps.tile([P, 2, S], F32, tag="pp2", name="pp2")
        pp1 = ps.tile([P, 2, S], F32, tag="pp1", name="pp1")
        for nt in range(2):
            for dt in range(2):
                nc.tensor.matmul(
                    pp2[:, nt, :],
                    w2t[dt][:, nt * P : (nt + 1) * P],
                    xt[dt][:],
                    start=(dt == 0),
                    stop=(dt == 1),
                    skip_group_check=True,
                )
        for nt in range(2):
            for dt in range(2):
                nc.tensor.matmul(
                    pp1[:, nt, :],
                    w1t[dt][:, nt * P : (nt + 1) * P],
                    xt[dt][:],
                    start=(dt == 0),
                    stop=False,
                    skip_group_check=True,
                )
            nc.tensor.matmul(
                pp1[:, nt, :],
                cbT[:, nt * P : (nt + 1) * P],
                ones[:],
                start=False,
                stop=True,
                skip_group_check=True,
            )

        # ---- seeds straight from PSUM, T2 first (feeds the longer chain)
        s2t = sb.tile([P, 2, S], BF16, tag="s2t", name="s2t")
        c2t = sb.tile([P, 2, S], BF16, tag="c2t", name="c2t")
        s1t = sb.tile([P, 2, S], BF16, tag="s1t", name="s1t")
        c1t = sb.tile([P, 2, S], BF16, tag="c1t", name="c1t")
        nc.scalar.activation(s2t[:], pp2[:], AF.Sin, scale=A0)
        nc.scalar.activation(c2t[:], pp2[:], AF.Sin, bias=cw[:, 13:14], scale=A0)
        nc.scalar.activation(s1t[:], pp1[:], AF.Sin, scale=A0)
        nc.scalar.activation(c1t[:], pp1[:], AF.Sin, bias=cw[:, 13:14], scale=A0)

        # ---- ladder + folds on DVE, in first-MM-need order.  The 4a0
        # cos factors come from cos(4fz) = 1 - 8 h^2 (h = sin(2fz)/2), so
        # there is no cross-engine round-trip; only q1 = s1^2 runs on ACT
        # (its idle window after the seeds).
        q2 = sb.tile([P, 2, S], BF16, tag="q2", name="q2")
        h2 = sb.tile([P, 2, S], BF16, tag="h2", name="h2")
        q1 = sb.tile([P, 2, S], BF16, tag="q1", name="q1")
        h1 = sb.tile([P, 2, S], BF16, tag="h1", name="h1")
        c11 = sb.tile([P, 2, S], BF16, tag="c11", name="c11")
        fc0 = sb.tile([P, 2, S], BF16, tag="fc0", name="fc0")
        fp0 = sb.tile([P, 2, S], BF16, tag="fp0", name="fp0")
        fp1 = sb.tile([P, 2, S], BF16, tag="fp1", name="fp1")
        fc1 = sb.tile([P, 2, S], BF16, tag="fc1", name="fc1")

        V = nc.vector
        for nt in range(2):  # tier-1 folds first: earliest MM operands
            V.tensor_scalar_mul(fc0[:, nt, :], s2t[:, nt, :], cw[:, 0 + nt : 1 + nt])
        V.tensor_mul(q2[:], s2t[:], s2t[:])
        for nt in range(2):
            V.tensor_scalar_mul(fp0[:, nt, :], c2t[:, nt, :], cw[:, 0 + nt : 1 + nt])
        V.tensor_mul(h2[:], s2t[:], c2t[:])
        for nt in range(2):  # fp1 = ws2*(1-2 q2), fused
            V.tensor_scalar(
                fp1[:, nt, :], q2[:, nt, :],
                cw[:, 4 + nt : 5 + nt], cw[:, 2 + nt : 3 + nt],
                OP.mult, OP.add,
            )
        V.tensor_scalar(c12[:], q2[:], -2.0, 1.0, OP.mult, OP.add)
        for nt in range(2):
            V.tensor_scalar_mul(fc1[:, nt, :], h2[:, nt, :], cw[:, 2 + nt : 3 + nt])
        V.tensor_mul(q1[:], s1t[:], s1t[:])
        V.tensor_mul(h1[:], s1t[:], c1t[:])
        V.tensor_scalar(c11[:], q1[:], -2.0, 1.0, OP.mult, OP.add)
        V.tensor_mul(u2[:], h2[:], h2[:])
        V.tensor_mul(g2[:], h2[:], c12[:])
        for nt in range(2):  # fp2 = K2*aw*(1 - 8 u2), fused
            V.tensor_scalar(
                fp2[:, nt, :], u2[:, nt, :],
                cw[:, 8 + nt : 9 + nt], cw[:, 10 + nt : 11 + nt],
                OP.mult, OP.add,
            )
        for nt in range(2):
            V.tensor_scalar_mul(fc2[:, nt, :], g2[:, nt, :], cw[:, 6 + nt : 7 + nt])
        V.tensor_mul(u1[:], h1[:], h1[:])
        V.tensor_mul(g1[:], h1[:], c11[:])
        V.tensor_scalar(c21[:], u1[:], -8.0, 1.0, OP.mult, OP.add)

        # Keep-busy bridges anchored on early-ready seeds fill the PE
        # operand-wait gap between the projections and the first burst.
        def bridge(anchor):
            nc.tensor.matmul(
                dps[:], anchor[:, 0, 0:P], dmy[:], start=False, stop=False,
                skip_group_check=True,
            )

        bridge(s2t)
        bridge(s2t)
        bridge(c2t)

        # ---- attention logits Apre^T[j,i]: 6 product pairs x 2 nt x 2 jt
        ap_ps = [
            ps.tile([P, S], F32, tag=f"apre{jt}", name=f"apre{jt}") for jt in range(2)
        ]
        pairs = (
            (fp0, s1t), (fc0, c1t),
            (fp1, h1), (fc1, c11),
        )
        for k, (stat, mov) in enumerate(pairs):
            for nt in range(2):
                for jt in range(2):
                    idx = k * 2 + nt
                    nc.tensor.matmul(
                        ap_ps[jt][:],
                        stat[:, nt, jt * P : (jt + 1) * P],
                        mov[:, nt, :],
                        start=(idx == 0),
                        stop=(idx == 7),
                        skip_group_check=True,
                    )

        # A^T = 0.5 + 0.5*tanh(z/2 + att_b/2); the tanh part goes through
        # the output matmul against XH2 = X/2, the 0.5 offset becomes a
        # ones-stationary colsum matmul.  Column halves so the first
        # output matmul can start after two ACT ops.
        at = [sb.tile([P, S], BF16, tag=f"at{jt}", name=f"at{jt}") for jt in range(2)]
        for jt in range(2):
            nc.scalar.activation(
                at[jt][:],
                ap_ps[jt][:],
                AF.Tanh,
                bias=cw[:, 12:13],
                scale=0.5,
            )

        # ---- out[i,d] = sum_j (0.5 + 0.5 th^T[j,i]) * X[j,d]
        #              = sum_j ones*XH2 + sum_j th^T[j,i]*XH2[j,d]
        for it in range(2):
            o_ps = ps.tile([P, D], F32, tag=f"ops{it}", name=f"ops{it}")
            for jt in range(2):
                nc.tensor.matmul(
                    o_ps[:],
                    ones[:, it * P : (it + 1) * P],
                    xh[jt][:],
                    start=(jt == 0),
                    stop=False,
                    skip_group_check=True,
                )
            for jt in range(2):
                nc.tensor.matmul(
                    o_ps[:],
                    at[jt][:, it * P : (it + 1) * P],
                    xh[jt][:],
                    start=False,
                    stop=(jt == 1),
                    skip_group_check=True,
                )
            oc = sb.tile([P, D], BF16, tag=f"oc{it}", name=f"oc{it}")
            if it == 0:
                nc.vector.tensor_copy(oc[:], o_ps[:])
                nc.sync.dma_start(out=out_d[0:P, :], in_=oc[:])
            else:
                nc.scalar.activation(oc[:], o_ps[:], AF.Copy)
                nc.scalar.dma_start(out=out_d[P : 2 * P, :], in_=oc[:])

    nc.finalize()
    _nc_cache["nc"] = nc
    return nc


def _host_prep(X, Wx_w, Wx_b, Wxhat_w, Wxhat_b, att_w, att_b):
    bf = ml_dtypes.bfloat16
    w1t = np.ascontiguousarray(Wx_w.T).astype(bf)
    w2t = np.ascontiguousarray(Wxhat_w.T).astype(bf)
    cb1 = (Wx_b + Wxhat_b).astype(np.float32).reshape(1, N).astype(bf)
    aw = att_w.astype(np.float32)
    ws = [COEFS[0] * aw, 2 * COEFS[1] * aw]
    cw = np.zeros((P, 14), np.float32)
    for nt in range(2):
        sl = slice(nt * P, (nt + 1) * P)
        cw[:, 0 + nt] = ws[0][sl]
        cw[:, 2 + nt] = ws[1][sl]
        cw[:, 4 + nt] = -2.0 * ws[1][sl]
    cw[:, 12] = 0.5 * float(np.asarray(att_b).reshape(-1)[0])
    cw[:, 13] = np.pi / 2
    shared = {"W1T": w1t, "W2T": w2t, "CW": cw, "CB1": cb1}
    in_maps = []
    for b in range(B):
        xb = np.ascontiguousarray(X[b], dtype=np.float32)
        in_maps.append(
            {
                "XH2": (0.5 * xb).astype(bf),
                "XT": np.ascontiguousarray(xb.T).astype(bf),
                **shared,
            }
        )
    return in_maps


def run(inputs, trace=False):
    nc = _build_nc()
    in_maps = _host_prep(**inputs)
    res = run_bass_kernel_spmd(nc, in_maps, core_ids=list(range(NCORES)), trace=trace)
    out = np.stack(
        [np.asarray(res.results[i]["out"]).astype(np.float32) for i in range(NCORES)],
        axis=0,
    )
    return out, res.exec_time_ns


def kernel(**inputs):
    out, _ = run(inputs, trace=False)
    return out


# revision 31
# speedup vs baseline: 1.1082x; 1.0145x over previous
"""Trainium2 Bass kernel for nn_AttentionLSTM (B=8, S=256, D=256, N=256).

Math:
  Wx  = X @ Wx_w.T + Wx_b            [B,S,N]
  Wxh = X @ Wxhat_w.T + Wxhat_b      [B,S,N]
  A   = sigmoid(tanh(Wxh[:,None,:,:] + Wx[:,:,None,:]) @ att_w + att_b)  [B,S,S]
  out = A @ X                         [B,S,D]

Strategy: data-parallel over batch (1 batch per NeuronCore, 8 cores).
The [S,S,N] tanh tensor is never materialized: tanh(t) ~ sum_m K_m
sin(2^m a0 t), each sine separating via angle addition into two rank-N
matmuls, 24 bf16 MMs total on the TensorEngine.

Structure (one act-table load for the whole kernel, set 18 has sin+tanh
+square):
- cb = Wx_b + Wxhat_b is folded into the T2 PSUM by one extra rank-1
  accumulating matmul per nt (stationary row 0 = cb, moving = ones), so
  all four ACT seeds are fused [128, 2, S] ops with constant/no bias.
- Only s/c come from the sin table; sin(2a0 z) = s*c and sin(4a0 z) =
  (s*c)*c1 ladders on DVE with the 2x/4x absorbed into fold coefs.
  The ladder squares (q1, k2, k1) run on ACT (Square) in its idle
  window between the seeds and the output activations.
- The output sigmoid is 0.5 + 0.5*tanh(z/2): tanh is in the sin set, so
  no second table load.  The affine is exact: X is pre-halved on the
  host (XH2 = X/2) and the 0.5*colsum(X) term is one extra
  ones-stationary matmul accumulated into the output PSUM.
- Input DMAs split across both HWDGE rings (SP + ACT) pairwise in need
  order; gpsimd SWDGE only carries memsets and the late xh1 load (its
  elementwise ops contend with DVE on the shared SBUF port - measured
  2-3x slowdown on both - so everything pointwise stays on DVE).
- Dummy/bridge matmuls keep the PE HAM activity window continuously
  busy so the clock un-throttles to 2.4 GHz before the attention
  bursts (v1 measured no un-throttle at all: every MM ran at 1.2 GHz).
"""

from contextlib import ExitStack

import math

import ml_dtypes
import numpy as np

import concourse.bacc as bacc
import concourse.bass as bass
import concourse.mybir as mybir
import concourse.tile as tile
from concourse.bass_utils import run_bass_kernel_spmd

F32 = mybir.dt.float32
BF16 = mybir.dt.bfloat16
AF = mybir.ActivationFunctionType
OP = mybir.AluOpType

B, S, D, N = 8, 256, 256, 256
NCORES = 8
P = 128

# tanh(t) ~ K0 sin(a0 t) + K1 sin(2 a0 t), end-to-end LSQ
A0 = 0.42
COEFS = (0.11045, 0.951604)

_nc_cache = {}


def _build_nc():
    if "nc" in _nc_cache:
        return _nc_cache["nc"]
    nc = bacc.Bacc()

    xt_d = nc.declare_dram_parameter("XT", [D, S], BF16, isOutput=False)
    xh_d = nc.declare_dram_parameter("XH2", [S, D], BF16, isOutput=False)  # X/2
    w1t_d = nc.declare_dram_parameter("W1T", [D, N], BF16, isOutput=False)
    w2t_d = nc.declare_dram_parameter("W2T", [D, N], BF16, isOutput=False)
    cb1_d = nc.declare_dram_parameter("CB1", [1, N], BF16, isOutput=False)
    # packed per-partition constants, cols:
    #   0:2   ws1[nt] = K0*att_w
    #   2:4   ws2[nt] = 2*K1*att_w
    #   4:6   -2*ws2[nt]
    #   6:8   ws3[nt] = 4*K2*att_w
    #   8:10  2*ws3[nt]
    #   10:12 -ws3[nt]
    #   12    att_b/2   (bias for the tanh-form sigmoid)
    #   13    pi/2
    cw_d = nc.declare_dram_parameter("CW", [P, 14], F32, isOutput=False)
    out_d = nc.declare_dram_parameter("out", [S, D], BF16, isOutput=True)

    with tile.TileContext(nc) as tc, ExitStack() as ctx:
        sb = ctx.enter_context(tc.tile_pool(name="sb", bufs=1))
        ps = ctx.enter_context(tc.tile_pool(name="ps", bufs=1, space="PSUM"))

        # Set 18 = silu_and_others: sin + tanh + square. The only load.
        nc.scalar.add_instruction(
            mybir.InstLoadActFuncSet(
                act_func_set_id=18, name=nc.get_next_instruction_name()
            )
        )

        xt = [sb.tile([P, S], BF16, tag=f"xt{i}", name=f"xt{i}") for i in range(2)]
        xh = [sb.tile([P, D], BF16, tag=f"xh{i}", name=f"xh{i}") for i in range(2)]
        w1t = [sb.tile([P, N], BF16, tag=f"w1t{i}", name=f"w1t{i}") for i in range(2)]
        w2t = [sb.tile([P, N], BF16, tag=f"w2t{i}", name=f"w2t{i}") for i in range(2)]
        cw = sb.tile([P, 14], F32, tag="cw", name="cw")
        cbR = sb.tile([1, N], BF16, tag="cbR", name="cbR")
        ones = sb.tile([P, S], BF16, tag="ones", name="ones")
        dmy = sb.tile([P, 4 * P], BF16, tag="dmy", name="dmy")

        # gpsimd: memsets (cbT before its CB1 row DMA) + small SWDGE loads.
        nc.gpsimd.memset(dmy[:], 0.0)
        nc.gpsimd.memset(ones[:], 1.0)
        nc.gpsimd.dma_start(out=xh[1][:], in_=xh_d[P : 2 * P, :])

        # SP HWDGE ring, in need order.
        nc.sync.dma_start(out=cbR[:], in_=cb1_d[:, :])
        nc.sync.dma_start(out=xt[0][:], in_=xt_d[0:P, :])
        nc.sync.dma_start(out=w2t[1][:], in_=w2t_d[P : 2 * P, :])
        nc.sync.dma_start(out=w1t[1][:], in_=w1t_d[P : 2 * P, :])
        nc.sync.dma_start(out=cw[:], in_=cw_d[:, :])
        nc.sync.dma_start(out=xh[0][:], in_=xh_d[0:P, :])

        # ACT HWDGE ring, in parallel: the partner operand of each pair.
        nc.scalar.dma_start(out=w2t[0][:], in_=w2t_d[0:P, :])
        nc.scalar.dma_start(out=xt[1][:], in_=xt_d[P : 2 * P, :])
        nc.scalar.dma_start(out=w1t[0][:], in_=w1t_d[0:P, :])

        dps = ps.tile([P, 4 * P], F32, tag="dps", name="dps")
        for i in range(6):
            nc.tensor.matmul(
                dps[:], dmy[:, 0:P], dmy[:], start=(i == 0), stop=False,
                skip_group_check=True,
            )

        # ---- projections, dt-major to match DMA arrival order.  T2 and T1
        # in SEPARATE PSUM tiles: tile-granular dependency tracking must not
        # make the T2 seeds wait for the T1 writers.
        pp2 = ps.tile([P, 2, S], F32, tag="pp2", name="pp2")
        pp1 = ps.tile([P, 2, S], F32, tag="pp1", name="pp1")
        for nt in range(2):
            for dt in range(2):
                nc.tensor.matmul(
                    pp2[:, nt, :],
                    w2t[dt][:, nt * P : (nt + 1) * P],
                    xt[dt][:],
                    start=(dt == 0),
                    stop=(dt == 1),
                    skip_group_check=True,
                )
        for nt in range(2):
            nc.tensor.matmul(
                pp1[:, nt, :],
                cbR[:, nt * P : (nt + 1) * P],
                ones[0:1, :],
                start=True,
                stop=False,
                skip_group_check=True,
            )
            for dt in range(2):
                nc.tensor.matmul(
                    pp1[:, nt, :],
                    w1t[dt][:, nt * P : (nt + 1) * P],
                    xt[dt][:],
                    start=False,
                    stop=(dt == 1),
                    skip_group_check=True,
                )

        # ---- seeds straight from PSUM, T2 first (feeds the longer chain)
        s2t = sb.tile([P, 2, S], BF16, tag="s2t", name="s2t")
        c2t = sb.tile([P, 2, S], BF16, tag="c2t", name="c2t")
        s1t = sb.tile([P, 2, S], BF16, tag="s1t", name="s1t")
        c1t = sb.tile([P, 2, S], BF16, tag="c1t", name="c1t")
        nc.scalar.activation(s2t[:], pp2[:], AF.Sin, scale=A0)
        nc.scalar.activation(c2t[:], pp2[:], AF.Sin, bias=cw[:, 13:14], scale=A0)
        nc.scalar.activation(s1t[:], pp1[:], AF.Sin, scale=A0)
        nc.scalar.activation(c1t[:], pp1[:], AF.Sin, bias=cw[:, 13:14], scale=A0)

        # ---- ladder + folds on DVE, in first-MM-need order.  The 4a0
        # cos factors come from cos(4fz) = 1 - 8 h^2 (h = sin(2fz)/2), so
        # there is no cross-engine round-trip; only q1 = s1^2 runs on ACT
        # (its idle window after the seeds).
        q2 = sb.tile([P, 2, S], BF16, tag="q2", name="q2")
        h2 = sb.tile([P, 2, S], BF16, tag="h2", name="h2")
        q1 = sb.tile([P, 2, S], BF16, tag="q1", name="q1")
        h1 = sb.tile([P, 2, S], BF16, tag="h1", name="h1")
        c11 = sb.tile([P, 2, S], BF16, tag="c11", name="c11")
        fc0 = sb.tile([P, 2, S], BF16, tag="fc0", name="fc0")
        fp0 = sb.tile([P, 2, S], BF16, tag="fp0", name="fp0")
        fp1 = sb.tile([P, 2, S], BF16, tag="fp1", name="fp1")
        fc1 = sb.tile([P, 2, S], BF16, tag="fc1", name="fc1")

        V = nc.vector
        for nt in range(2):  # tier-1 folds first: earliest MM operands
            V.tensor_scalar_mul(fc0[:, nt, :], s2t[:, nt, :], cw[:, 0 + nt : 1 + nt])
        V.tensor_mul(q2[:], s2t[:], s2t[:])
        for nt in range(2):
            V.tensor_scalar_mul(fp0[:, nt, :], c2t[:, nt, :], cw[:, 0 + nt : 1 + nt])
        V.tensor_mul(h2[:], s2t[:], c2t[:])
        for nt in range(2):  # fp1 = ws2*(1-2 q2), fused
            V.tensor_scalar(
                fp1[:, nt, :], q2[:, nt, :],
                cw[:, 4 + nt : 5 + nt], cw[:, 2 + nt : 3 + nt],
                OP.mult, OP.add,
            )
        V.tensor_scalar(c12[:], q2[:], -2.0, 1.0, OP.mult, OP.add)
        for nt in range(2):
            V.tensor_scalar_mul(fc1[:, nt, :], h2[:, nt, :], cw[:, 2 + nt : 3 + nt])
        V.tensor_mul(q1[:], s1t[:], s1t[:])
        V.tensor_mul(h1[:], s1t[:], c1t[:])
        V.tensor_scalar(c11[:], q1[:], -2.0, 1.0, OP.mult, OP.add)
        V.tensor_mul(u2[:], h2[:], h2[:])
        V.tensor_mul(g2[:], h2[:], c12[:])
        for nt in range(2):  # fp2 = K2*aw*(1 - 8 u2), fused
            V.tensor_scalar(
                fp2[:, nt, :], u2[:, nt, :],
                cw[:, 8 + nt : 9 + nt], cw[:, 10 + nt : 11 + nt],
                OP.mult, OP.add,
            )
        for nt in range(2):
            V.tensor_scalar_mul(fc2[:, nt, :], g2[:, nt, :], cw[:, 6 + nt : 7 + nt])
        V.tensor_mul(u1[:], h1[:], h1[:])
        V.tensor_mul(g1[:], h1[:], c11[:])
        V.tensor_scalar(c21[:], u1[:], -8.0, 1.0, OP.mult, OP.add)

        # Keep-busy bridges anchored on early-ready seeds fill the PE
        # operand-wait gap between the projections and the first burst.
        def bridge(anchor):
            nc.tensor.matmul(
                dps[:], anchor[:, 0, 0:P], dmy[:], start=False, stop=False,
                skip_group_check=True,
            )

        bridge(s2t)
        bridge(s2t)
        bridge(c2t)

        # ---- attention logits Apre^T[j,i]: 6 product pairs x 2 nt x 2 jt
        ap_ps = [
            ps.tile([P, S], F32, tag=f"apre{jt}", name=f"apre{jt}") for jt in range(2)
        ]
        pairs = (
            (fp0, s1t), (fc0, c1t),
            (fp1, h1), (fc1, c11),
        )
        for k, (stat, mov) in enumerate(pairs):
            for nt in range(2):
                for jt in range(2):
                    idx = k * 2 + nt
                    nc.tensor.matmul(
                        ap_ps[jt][:],
                        stat[:, nt, jt * P : (jt + 1) * P],
                        mov[:, nt, :],
                        start=(idx == 0),
                        stop=(idx == 7),
                        skip_group_check=True,
                    )

        # A^T = 0.5 + 0.5*tanh(z/2 + att_b/2); the tanh part goes through
        # the output matmul against XH2 = X/2, the 0.5 offset becomes a
        # ones-stationary colsum matmul.  Column halves so the first
        # output matmul can start after two ACT ops.
        at = [sb.tile([P, S], BF16, tag=f"at{jt}", name=f"at{jt}") for jt in range(2)]
        for jt in range(2):
            nc.scalar.activation(
                at[jt][:],
                ap_ps[jt][:],
                AF.Tanh,
                bias=cw[:, 12:13],
                scale=0.5,
            )

        # ---- out[i,d] = sum_j (0.5 + 0.5 th^T[j,i]) * X[j,d]
        #              = sum_j ones*XH2 + sum_j th^T[j,i]*XH2[j,d]
        for it in range(2):
            o_ps = ps.tile([P, D], F32, tag=f"ops{it}", name=f"ops{it}")
            for jt in range(2):
                nc.tensor.matmul(
                    o_ps[:],
                    ones[:, it * P : (it + 1) * P],
                    xh[jt][:],
                    start=(jt == 0),
                    stop=False,
                    skip_group_check=True,
                )
            for jt in range(2):
                nc.tensor.matmul(
                    o_ps[:],
                    at[jt][:, it * P : (it + 1) * P],
                    xh[jt][:],
                    start=False,
                    stop=(jt == 1),
                    skip_group_check=True,
                )
            oc = sb.tile([P, D], BF16, tag=f"oc{it}", name=f"oc{it}")
            if it == 0:
                nc.vector.tensor_copy(oc[:], o_ps[:])
                nc.sync.dma_start(out=out_d[0:P, :], in_=oc[:])
            else:
                nc.scalar.activation(oc[:], o_ps[:], AF.Copy)
                nc.scalar.dma_start(out=out_d[P : 2 * P, :], in_=oc[:])

    nc.finalize()
    _nc_cache["nc"] = nc
    return nc


def _host_prep(X, Wx_w, Wx_b, Wxhat_w, Wxhat_b, att_w, att_b):
    bf = ml_dtypes.bfloat16
    w1t = np.ascontiguousarray(Wx_w.T).astype(bf)
    w2t = np.ascontiguousarray(Wxhat_w.T).astype(bf)
    cb1 = (Wx_b + Wxhat_b).astype(np.float32).reshape(1, N).astype(bf)
    aw = att_w.astype(np.float32)
    ws = [COEFS[0] * aw, 2 * COEFS[1] * aw]
    cw = np.zeros((P, 14), np.float32)
    for nt in range(2):
        sl = slice(nt * P, (nt + 1) * P)
        cw[:, 0 + nt] = ws[0][sl]
        cw[:, 2 + nt] = ws[1][sl]
        cw[:, 4 + nt] = -2.0 * ws[1][sl]
    cw[:, 12] = 0.5 * float(np.asarray(att_b).reshape(-1)[0])
    cw[:, 13] = np.pi / 2
    shared = {"W1T": w1t, "W2T": w2t, "CW": cw, "CB1": cb1}
    in_maps = []
    for b in range(B):
        xb = np.ascontiguousarray(X[b], dtype=np.float32)
        in_maps.append(
            {
                "XH2": (0.5 * xb).astype(bf),
                "XT": np.ascontiguousarray(xb.T).astype(bf),
                **shared,
            }
        )
    return in_maps


def run(inputs, trace=False):
    nc = _build_nc()
    in_maps = _host_prep(**inputs)
    res = run_bass_kernel_spmd(nc, in_maps, core_ids=list(range(NCORES)), trace=trace)
    out = np.stack(
        [np.asarray(res.results[i]["out"]).astype(np.float32) for i in range(NCORES)],
        axis=0,
    )
    return out, res.exec_time_ns


def kernel(**inputs):
    out, _ = run(inputs, trace=False)
    return out
